# revision 31
# baseline (speedup 1.0000x reference)
"""MultiHeadAttention TRN2 Bass kernel (v4, fp16 datapath).

Problem: S=2048, B=2, H=16, d_k=64, D=1024, fp32 interface.
  q = query @ Wq.T + bq ; k = key @ Wk.T + bk ; v = value @ Wv.T + bv
  score = einsum('qbhd,kbhd->qkbh', q, k) / 8 ; attn = softmax(score, axis=k)
  out = einsum('qkbh,kbhd->qbhd', attn, v) -> reshape -> @ Wo.T + bo

Sharding (8 cores): core c handles batch b = c//4 and heads [4*(c%4), 4*(c%4)+4).
Each core computes its partial output projection (tensor-parallel along the
head dim); the host sums the 4 partials per batch and adds the bias terms
(bv @ Wo.T + bo, the linear-foldable bias contributions).

Key techniques (see git-style history in the module docstrings of prior
versions):
  * fp16 device datapath (host casts inputs/weights; DMA bytes halve; PE
    speed identical to f32r at 1 cycle/row; PSUM accumulates fp32).
  * V head slots are 128 cols: [64 replicated ones | 64 value dims]. The PV
    matmul cost is free-dim bound, so the otherwise idle M columns produce
    the softmax denominator pre-broadcast across 64 partitions for free.
    Ones first: custom-DVE ops ignore input partition offsets, so the
    denominator must sit at partition 0 for the fast reciprocal.
  * Softmax normalize fully on DVE straight from PSUM: fast-reciprocal of
    pv[0:64] then one multiply into fp16 AC.
  * Score matmuls for a head pair use disjoint PE partition halves (K=64 at
    base 0 / 64) and co-run on the PE array.
  * DMA: weights ride the SP queue as whole-tensor transfers; x tiles ride
    the DVE queue (a parallel HWDGE issue stream — each dma_start costs
    ~0.65us of queue time, so a single queue serializes startup).
  * The kb loop emits PV(kb-1) before scores(kb) so the PE stream alternates
    deterministically; exp(kb) on ScalarE is the steady-state limiter.
  * Each qb's output projection drains through the next qb's kb loop (the
    drain queue) instead of bursting at the qb boundary.

Per-core device layout (host pre-transposes + casts, pure data layout):
  xqT/xkT/xvT : [D=1024, T=2048] fp16   input slices, feature-major
  wqT/wkT/wvT : [1024, 256] fp16        Wq[rows,:].T etc (lhsT tiles directly)
  woT         : [256, 1024] fp16        Wo[:, cols].T (rhs tiles directly)
  bqv/bkv     : [256] fp32              projection biases for q/k
  out         : [2048, 1024] fp16       partial output (token-major)
"""

import os

os.environ.setdefault("MYCRO_LOCAL_CACHE", "1")

import numpy as np

import concourse.bass as bass
import concourse.tile as tile
from concourse import bacc, bass_utils, mybir


def _install_ntff_hook():
    """Provide antenv.axon_hooks when the image lacks it, so trace=True can
    capture NTFF profiles through the axon tunnel. Degrades silently."""
    import contextlib
    import ctypes
    import sys

    if "antenv.axon_hooks" in sys.modules:
        return
    so_path = "/opt/axon/libaxon_pjrt.so"
    if not os.path.exists(so_path):
        return
    try:
        lib = ctypes.CDLL(so_path)
        if not hasattr(lib, "axon_start_nrt_profile"):
            return
        lib.axon_start_nrt_profile.argtypes = [
            ctypes.POINTER(ctypes.c_int64),
            ctypes.c_size_t,
        ]
        lib.axon_start_nrt_profile.restype = ctypes.c_int64
        lib.axon_stop_nrt_profile.argtypes = [ctypes.c_char_p]
        lib.axon_stop_nrt_profile.restype = ctypes.c_int64

        @contextlib.contextmanager
        def _hook(output_dir, device_ids):
            import jax

            jax.devices()
            if device_ids:
                ids = (ctypes.c_int64 * len(device_ids))(*device_ids)
                rc = lib.axon_start_nrt_profile(ids, len(device_ids))
            else:
                rc = lib.axon_start_nrt_profile(None, 0)
            if rc != 0:
                raise RuntimeError(f"axon_start_nrt_profile rc={rc}")
            try:
                yield
            finally:
                n = lib.axon_stop_nrt_profile(str(output_dir).encode())
                print(f"ntff profile: {n} file(s) -> {output_dir}")

        import types

        mod = types.ModuleType("antenv.axon_hooks")
        mod.get_axon_ntff_profile_hook = lambda: _hook
        mod.set_axon_ntff_profile_hook = lambda h: None
        sys.modules["antenv.axon_hooks"] = mod
    except Exception:
        pass


_install_ntff_hook()

F32 = mybir.dt.float32
FP16 = mybir.dt.float16
AF = mybir.ActivationFunctionType

S = 2048          # sequence length
B = 2             # batch
H = 16            # total heads
DK = 64           # head dim
D = 1024          # model dim
NCORES = 8
HL = H // (NCORES // B)   # heads per core = 4
HC = HL * DK              # head cols per core = 256
T = S                     # tokens per core (one batch element)
P = 128
QB = 512                  # q block (matmul free dim)
NKB = T // P              # 16 k blocks
NQB = T // QB             # 4 q blocks
VW = 2 * DK               # 128: [64 ones | 64 value dims] per head slot


def build_module():
    nc = bacc.Bacc("TRN2", target_bir_lowering=False, debug=False)

    NKC = D // P
    # weights arrive pre-arranged so each partition's data is one contiguous
    # 4KB run (128 big DMA descriptors instead of 1024 x 512B)
    xqT = nc.dram_tensor("xqT", [D, T], FP16, kind="ExternalInput").ap()
    xkT = nc.dram_tensor("xkT", [D, T], FP16, kind="ExternalInput").ap()
    xvT = nc.dram_tensor("xvT", [D, T], FP16, kind="ExternalInput").ap()
    wqT = nc.dram_tensor("wqT", [P, NKC, HC], FP16, kind="ExternalInput").ap()
    wkT = nc.dram_tensor("wkT", [P, 2, NKC, P], FP16, kind="ExternalInput").ap()
    wvT = nc.dram_tensor("wvT", [P, NKC, HC], FP16, kind="ExternalInput").ap()
    woT = nc.dram_tensor("woT", [P, HC // P, D], FP16, kind="ExternalInput").ap()
    bqv = nc.dram_tensor("bqv", [P, HC // P], F32, kind="ExternalInput").ap()
    bkv = nc.dram_tensor("bkv", [P, HC // P], F32, kind="ExternalInput").ap()
    out = nc.dram_tensor("out", [T, D], FP16, kind="ExternalOutput").ap()

    with tile.TileContext(nc) as tc:
        kernel_body(tc, xqT, xkT, xvT, wqT, wkT, wvT, woT, bqv, bkv, out)

    nc.compile()
    return nc


def kernel_body(tc, xqT, xkT, xvT, wqT, wkT, wvT, woT, bqv, bkv, out):
    nc = tc.nc
    NKC = D // P   # 8 contraction chunks for projections
    NXP = NKC // 2  # 4 kc-pair x tiles per (tensor, tb)

    with (
        tc.tile_pool(name="consts", bufs=1) as consts,
        tc.tile_pool(name="xs", bufs=14) as xs,
        tc.tile_pool(name="persist", bufs=1) as persist,
        tc.tile_pool(name="attn", bufs=6) as attn_pool,
        tc.tile_pool(name="small", bufs=4) as small,
        tc.tile_pool(name="outs", bufs=4) as outs,
        tc.tile_pool(name="ps_mm", bufs=2, space="PSUM") as ps_mm,
        tc.tile_pool(name="ps_sc", bufs=2, space="PSUM") as ps_sc,
        tc.tile_pool(name="ps_pv", bufs=2, space="PSUM") as ps_pv,
    ):
        # ------------- x tiles (DVE DMA queue — parallel to SP) -------------
        _xid = [0]

        def x_load(xT, tb, tag):
            """Load one token block's 8 kc chunks as 4 kc-pair tiles."""
            _xid[0] += 1
            ts = []
            for j in range(NXP):
                t = xs.tile([P, 2, QB], FP16, tag="x", name=f"{tag}_{_xid[0]}_{j}")
                nc.gpsimd.dma_start(
                    t,
                    xT.rearrange("(kc p) t -> p kc t", p=P)[
                        :, 2 * j : 2 * j + 2, tb * QB : (tb + 1) * QB
                    ],
                )
                ts.append(t)
            return ts

        def x_ap(xts, kc):
            return xts[kc // 2][:, kc % 2, :]

        # ------------- constants (SP DMA queue) -------------
        # wk arrives m-chunk-major and lands as two half DMAs so the very
        # first projection matmul only waits for the m0 half.
        wk_s = consts.tile([P, 2, NKC, P], FP16)
        nc.sync.dma_start(wk_s[:, 0], wkT[:, 0])
        nc.sync.dma_start(wk_s[:, 1], wkT[:, 1])
        bk_s = consts.tile([P, HC // P], F32)
        nc.sync.dma_start(bk_s, bkv)

        # ------------- persistent activations -------------
        QT = [persist.tile([P, T], FP16, name=f"QT{m}") for m in range(2)]
        KT = [persist.tile([P, T], FP16, name=f"KT{m}") for m in range(2)]
        V = persist.tile([P, NKB, HL * VW], FP16, name="V")
        AC = [persist.tile([P, T], FP16, name=f"AC{c}") for c in range(2)]

        # whole-tile memset: evacs overwrite the value cols, ones cols stay 1.0
        nc.vector.memset(V, 1.0)

        # ------------- projection emitters -------------
        def wk_ap(kc, m):
            return wk_s[:, m, kc, :]

        def wq_ap(kc, m):
            return wq_s[:, kc, m * P : (m + 1) * P]

        def proj_qk(xts, w_ap, b_s, dst, tag, tb, m):
            ps = ps_mm.tile([P, QB], F32, tag="mm", name=f"p_{tag}{m}{tb}")
            for kc in range(NKC):
                nc.tensor.matmul(
                    ps,
                    lhsT=w_ap(kc, m),
                    rhs=x_ap(xts, kc),
                    start=(kc == 0),
                    stop=(kc == NKC - 1),
                )
            nc.vector.tensor_scalar_add(
                dst[m][:, tb * QB : (tb + 1) * QB], ps, b_s[:, m : m + 1]
            )

        def proj_v_block(xts, tb, i):
            t128 = tb * (QB // P) + i
            ps = ps_mm.tile([P, HC], F32, tag="mm", name=f"p_v{t128}")
            for kc in range(NKC):
                nc.tensor.matmul(
                    ps,
                    lhsT=x_ap(xts, kc)[:, i * P : (i + 1) * P],
                    rhs=wv_s[:, kc, :],
                    start=(kc == 0),
                    stop=(kc == NKC - 1),
                )
            nc.vector.tensor_copy(
                V[:, t128].rearrange("p (h c) -> p h c", c=VW)[:, :, DK:],
                ps.rearrange("p (h c) -> p h c", c=DK),
            )

        def qk_units(xT, w_ap, b_s, dst, tag, tb, ms):
            """Zip units for one token block: shared x load + per-m jobs."""
            st = {}
            units = [lambda st=st, tb=tb: st.__setitem__("x", x_load(xT, tb, tag))]
            for m in ms:
                for kc in range(NKC):

                    def mk_mm(m=m, kc=kc, st=st, tb=tb):
                        if kc == 0:
                            st["ps", m] = ps_mm.tile(
                                [P, QB], F32, tag="mm", name=f"pz_{tag}{m}{tb}"
                            )
                        nc.tensor.matmul(
                            st["ps", m],
                            lhsT=w_ap(kc, m),
                            rhs=x_ap(st["x"], kc),
                            start=(kc == 0),
                            stop=(kc == NKC - 1),
                        )

                    units.append(mk_mm)

                def mk_evac(m=m, st=st, tb=tb):
                    nc.vector.tensor_scalar_add(
                        dst[m][:, tb * QB : (tb + 1) * QB],
                        st["ps", m],
                        b_s[:, m : m + 1],
                    )

                units.append(mk_evac)
            return units

        def v_units(tb):
            st = {}
            units = [lambda st=st, tb=tb: st.__setitem__("x", x_load(xvT, tb, "xv"))]
            for i in range(QB // P):
                units.append(lambda i=i, st=st, tb=tb: proj_v_block(st["x"], tb, i))
            return units

        # ------------- stage A: K/V/Q for token blocks 0-1 -------------
        # Covers attention kb 0..7 and q blocks 0..1; tb2-3 + the output
        # projections drain through the attention kb loop.
        xk01 = {tb: x_load(xkT, tb, "xk") for tb in (0, 1)}
        wv_s = consts.tile([P, NKC, HC], FP16)
        nc.sync.dma_start(wv_s, wvT)
        wq_s = consts.tile([P, NKC, HC], FP16)
        nc.sync.dma_start(wq_s, wqT)
        bq_s = consts.tile([P, HC // P], F32)
        nc.sync.dma_start(bq_s, bqv)
        for m in (0, 1):
            for tb in (0, 1):
                proj_qk(xk01[tb], wk_ap, bk_s, KT, "xk", tb, m)
        xv01 = {tb: x_load(xvT, tb, "xv") for tb in (0, 1)}
        for tb in (0, 1):
            for i in range(QB // P):
                proj_v_block(xv01[tb], tb, i)
        xq01 = {tb: x_load(xqT, tb, "xq") for tb in (0, 1)}
        wo_s = consts.tile([P, HC // P, D], FP16)
        nc.sync.dma_start(wo_s, woT)
        for m in (0, 1):
            for tb in (0, 1):
                proj_qk(xq01[tb], wq_ap, bq_s, QT, "xq", tb, m)

        # remaining projections, ordered by when attention needs them:
        # tb2 by kb8, tb3 by kb12 (m0/hp0), m1 chunks by kb16+ (hp1),
        # Q tb2/tb3 by qb2/qb3.
        zip_units = (
            qk_units(xkT, wk_ap, bk_s, KT, "xk", 2, (0, 1))
            + v_units(2)
            + qk_units(xkT, wk_ap, bk_s, KT, "xk", 3, (0, 1))
            + v_units(3)
            + qk_units(xqT, wq_ap, bq_s, QT, "xq", 2, (0, 1))
            + qk_units(xqT, wq_ap, bq_s, QT, "xq", 3, (0, 1))
        )
        zq = list(zip_units)[::-1]  # pop from end

        def drain(n):
            for _ in range(n):
                if zq:
                    zq.pop()()

        def oproj_units(qb):
            """Output projection for one q block as drainable units."""
            units = []
            for i in range(QB // P):
                t128 = qb * (QB // P) + i
                st = {}

                def mk_mm(t128=t128, st=st, n=0):
                    ps = ps_mm.tile([P, 512], F32, tag="mm", name=f"ps_o{t128}{n}")
                    for c in range(2):
                        nc.tensor.matmul(
                            ps,
                            lhsT=AC[c][:, t128 * P : (t128 + 1) * P],
                            rhs=wo_s[:, c, n * 512 : (n + 1) * 512],
                            start=(c == 0),
                            stop=(c == 1),
                        )
                    st[n] = ps

                def mk_out(t128=t128, st=st, n=0):
                    ob = outs.tile([P, 512], FP16, tag="ob", name=f"ob_{t128}_{n}")
                    nc.vector.tensor_copy(ob, st[n])
                    nc.sync.dma_start(
                        out[t128 * P : (t128 + 1) * P, n * 512 : (n + 1) * 512], ob
                    )

                for n in range(2):
                    units.append(lambda f=mk_mm, n=n: f(n=n))
                    units.append(lambda f=mk_out, n=n: f(n=n))
            return units

        # ---------------- attention ----------------
        # Head pairs (2*hp, 2*hp+1) run their score matmuls concurrently on
        # disjoint PE row groups (K=64 each, base partitions 0 / 64).
        #
        # One flat software pipeline across all (qb, hp) iterations with a
        # 2-kb lag: PV for block kb issues after the score pair for kb+2, so
        # when the PE reaches a pv pair its exp has long completed — the PE
        # stream never stalls on a just-finished exp (ScalarE stays the
        # steady-state limiter), including across hp/qb boundaries. The
        # normalize for an hp and the O-projection hand-off ride the flush
        # of that hp's last k block, two slots into the next hp.
        def emit_pv(e):
            qb, hp, kb, at, pv0, pv1 = e
            m, h0, h1 = hp, 2 * hp, 2 * hp + 1
            nc.tensor.matmul(
                pv0,
                lhsT=V[:, kb, VW * h0 : VW * (h0 + 1)],
                rhs=at[:, :QB],
                start=(kb == 0),
                stop=(kb == NKB - 1),
            )
            nc.tensor.matmul(
                pv1,
                lhsT=V[:, kb, VW * h1 : VW * (h1 + 1)],
                rhs=at[:, QB:],
                start=(kb == 0),
                stop=(kb == NKB - 1),
            )
            if kb == NKB - 1:
                for h, pv in ((h0, pv0), (h1, pv1)):
                    off = 64 * (h % 2)
                    # rows 0-63 of pv: softmax denominator replicated across
                    # 64 partitions (ones cols of V); rows 64-127: the head
                    # values. One DVE fast-reciprocal + one multiply from
                    # PSUM. (The custom-DVE reciprocal ignores input
                    # partition offsets — its input must sit at partition 0.)
                    rcp_bc = small.tile(
                        [DK, QB], F32, tag="rcp", name=f"rcp_{qb}_{h}"
                    )
                    nc.vector.reciprocal_approx_fast(rcp_bc, pv[:DK, :])
                    nc.vector.tensor_mul(
                        AC[m][off : off + DK, qb * QB : (qb + 1) * QB],
                        pv[DK:P, :],
                        rcp_bc,
                    )
                if hp == 1:
                    # this qb's AC is complete: queue its output projection
                    zq.extend(oproj_units(qb)[::-1])

        pend = []
        for qb in range(NQB):
            for hp in range(2):
                m = hp  # heads (2*hp, 2*hp+1) live in QT/KT chunk m
                pv0 = ps_pv.tile([P, QB], F32, tag="pv", name=f"pv_{qb}_{hp}0")
                pv1 = ps_pv.tile([P, QB], F32, tag="pv", name=f"pv_{qb}_{hp}1")
                for kb in range(NKB):
                    sc = ps_sc.tile(
                        [P, 2 * QB], F32, tag="sc", name=f"sc_{qb}_{hp}_{kb}"
                    )
                    nc.tensor.matmul(
                        sc[:, :QB],
                        lhsT=KT[m][0:DK, kb * P : (kb + 1) * P],
                        rhs=QT[m][0:DK, qb * QB : (qb + 1) * QB],
                        start=True,
                        stop=True,
                    )
                    nc.tensor.matmul(
                        sc[:, QB:],
                        lhsT=KT[m][DK:P, kb * P : (kb + 1) * P],
                        rhs=QT[m][DK:P, qb * QB : (qb + 1) * QB],
                        start=True,
                        stop=True,
                    )
                    at = attn_pool.tile(
                        [P, 2 * QB], FP16, tag="at", name=f"at_{qb}_{hp}_{kb}"
                    )
                    nc.scalar.activation(at, sc, AF.Exp, scale=0.125)
                    pend.append((qb, hp, kb, at, pv0, pv1))
                    if len(pend) > 2:
                        emit_pv(pend.pop(0))
                    drain(5 if qb == 0 else 2)
        for e in pend:
            emit_pv(e)

        drain(len(zip_units) + 64)


_module_cache = None


def get_module():
    global _module_cache
    if _module_cache is None:
        _module_cache = build_module()
    return _module_cache


def shard_inputs(query, key, value, Wq, bq, Wk, bk, Wv, bv, Wo, bo):
    """Build the 8 per-core input maps (host-side layout transforms only)."""
    f = np.float32
    h = np.float16
    xT = {}
    for b in range(B):
        xT["q", b] = np.ascontiguousarray(np.asarray(query, f)[:, b, :].T.astype(h))
        xT["k", b] = np.ascontiguousarray(np.asarray(key, f)[:, b, :].T.astype(h))
        xT["v", b] = np.ascontiguousarray(np.asarray(value, f)[:, b, :].T.astype(h))
    Wq, Wk, Wv, Wo = (np.asarray(w, f) for w in (Wq, Wk, Wv, Wo))
    bq, bk = np.asarray(bq, f), np.asarray(bk, f)

    def w_arr(WT):
        # [D, HC] -> [P, NKC, HC]: partition-contiguous for big DMA descriptors
        kc = WT.shape[0] // P
        return np.ascontiguousarray(WT.reshape(kc, P, -1).transpose(1, 0, 2).astype(h))

    def b_arr(bv_):
        return np.ascontiguousarray(bv_.reshape(-1, P).T)

    in_maps = []
    for c in range(NCORES):
        b, hg = c // (NCORES // B), c % (NCORES // B)
        cols = slice(HC * hg, HC * (hg + 1))
        in_maps.append(
            {
                "xqT": xT["q", b],
                "xkT": xT["k", b],
                "xvT": xT["v", b],
                "wqT": w_arr(Wq[cols, :].T),
                "wkT": np.ascontiguousarray(
                    Wk[cols, :].T.reshape(8, P, 2, P).transpose(1, 2, 0, 3).astype(h)
                ),
                "wvT": w_arr(Wv[cols, :].T),
                "woT": w_arr(Wo[:, cols].T),
                "bqv": b_arr(bq[cols]),
                "bkv": b_arr(bk[cols]),
            }
        )
    return in_maps


def kernel(query, key, value, Wq, bq, Wk, bk, Wv, bv, Wo, bo, trace=False):
    nc = get_module()
    in_maps = shard_inputs(query, key, value, Wq, bq, Wk, bk, Wv, bv, Wo, bo)
    res = bass_utils.run_bass_kernel_spmd(
        nc, in_maps, core_ids=list(range(NCORES)), trace=trace
    )
    f = np.float32
    bias_term = np.asarray(bv, f) @ np.asarray(Wo, f).T + np.asarray(bo, f)
    output = np.empty((S, B, D), f)
    for b in range(B):
        acc = res.results[4 * b]["out"].astype(f)
        for c in range(4 * b + 1, 4 * b + 4):
            acc = acc + res.results[c]["out"].astype(f)
        output[:, b, :] = acc + bias_term
    if trace:
        kernel.last_results = res
    return output


# revision 34
# speedup vs baseline: 1.0204x; 1.0204x over previous
"""MultiHeadAttention TRN2 Bass kernel (v4, fp16 datapath).

Problem: S=2048, B=2, H=16, d_k=64, D=1024, fp32 interface.
  q = query @ Wq.T + bq ; k = key @ Wk.T + bk ; v = value @ Wv.T + bv
  score = einsum('qbhd,kbhd->qkbh', q, k) / 8 ; attn = softmax(score, axis=k)
  out = einsum('qkbh,kbhd->qbhd', attn, v) -> reshape -> @ Wo.T + bo

Sharding (8 cores): core c handles batch b = c//4 and heads [4*(c%4), 4*(c%4)+4).
Each core computes its partial output projection (tensor-parallel along the
head dim); the host sums the 4 partials per batch and adds the bias terms
(bv @ Wo.T + bo, the linear-foldable bias contributions).

Key techniques (see git-style history in the module docstrings of prior
versions):
  * fp16 device datapath (host casts inputs/weights; DMA bytes halve; PE
    speed identical to f32r at 1 cycle/row; PSUM accumulates fp32).
  * V head slots are 128 cols: [64 replicated ones | 64 value dims]. The PV
    matmul cost is free-dim bound, so the otherwise idle M columns produce
    the softmax denominator pre-broadcast across 64 partitions for free.
    Ones first: custom-DVE ops ignore input partition offsets, so the
    denominator must sit at partition 0 for the fast reciprocal.
  * Softmax normalize fully on DVE straight from PSUM: fast-reciprocal of
    pv[0:64] then one multiply into fp16 AC.
  * Score matmuls for a head pair use disjoint PE partition halves (K=64 at
    base 0 / 64) and co-run on the PE array.
  * DMA: weights ride the SP queue as whole-tensor transfers; x tiles ride
    the DVE queue (a parallel HWDGE issue stream — each dma_start costs
    ~0.65us of queue time, so a single queue serializes startup).
  * The kb loop emits PV(kb-1) before scores(kb) so the PE stream alternates
    deterministically; exp(kb) on ScalarE is the steady-state limiter.
  * Each qb's output projection drains through the next qb's kb loop (the
    drain queue) instead of bursting at the qb boundary.

Per-core device layout (host pre-transposes + casts, pure data layout):
  xqT/xkT/xvT : [D=1024, T=2048] fp16   input slices, feature-major
  wqT/wkT/wvT : [1024, 256] fp16        Wq[rows,:].T etc (lhsT tiles directly)
  woT         : [256, 1024] fp16        Wo[:, cols].T (rhs tiles directly)
  bqv/bkv     : [256] fp32              projection biases for q/k
  out         : [2048, 1024] fp16       partial output (token-major)
"""

import os

os.environ.setdefault("MYCRO_LOCAL_CACHE", "1")

import numpy as np

import concourse.bass as bass
import concourse.tile as tile
from concourse import bacc, bass_utils, mybir


def _install_ntff_hook():
    """Provide antenv.axon_hooks when the image lacks it, so trace=True can
    capture NTFF profiles through the axon tunnel. Degrades silently."""
    import contextlib
    import ctypes
    import sys

    if "antenv.axon_hooks" in sys.modules:
        return
    so_path = "/opt/axon/libaxon_pjrt.so"
    if not os.path.exists(so_path):
        return
    try:
        lib = ctypes.CDLL(so_path)
        if not hasattr(lib, "axon_start_nrt_profile"):
            return
        lib.axon_start_nrt_profile.argtypes = [
            ctypes.POINTER(ctypes.c_int64),
            ctypes.c_size_t,
        ]
        lib.axon_start_nrt_profile.restype = ctypes.c_int64
        lib.axon_stop_nrt_profile.argtypes = [ctypes.c_char_p]
        lib.axon_stop_nrt_profile.restype = ctypes.c_int64

        @contextlib.contextmanager
        def _hook(output_dir, device_ids):
            import jax

            jax.devices()
            if device_ids:
                ids = (ctypes.c_int64 * len(device_ids))(*device_ids)
                rc = lib.axon_start_nrt_profile(ids, len(device_ids))
            else:
                rc = lib.axon_start_nrt_profile(None, 0)
            if rc != 0:
                raise RuntimeError(f"axon_start_nrt_profile rc={rc}")
            try:
                yield
            finally:
                n = lib.axon_stop_nrt_profile(str(output_dir).encode())
                print(f"ntff profile: {n} file(s) -> {output_dir}")

        import types

        mod = types.ModuleType("antenv.axon_hooks")
        mod.get_axon_ntff_profile_hook = lambda: _hook
        mod.set_axon_ntff_profile_hook = lambda h: None
        sys.modules["antenv.axon_hooks"] = mod
    except Exception:
        pass


_install_ntff_hook()

F32 = mybir.dt.float32
FP16 = mybir.dt.float16
AF = mybir.ActivationFunctionType

S = 2048          # sequence length
B = 2             # batch
H = 16            # total heads
DK = 64           # head dim
D = 1024          # model dim
NCORES = 8
HL = H // (NCORES // B)   # heads per core = 4
HC = HL * DK              # head cols per core = 256
T = S                     # tokens per core (one batch element)
P = 128
QB = 512                  # q block (matmul free dim)
NKB = T // P              # 16 k blocks
NQB = T // QB             # 4 q blocks
VW = 2 * DK               # 128: [64 ones | 64 value dims] per head slot


def build_module():
    nc = bacc.Bacc("TRN2", target_bir_lowering=False, debug=False)

    NKC = D // P
    # weights arrive pre-arranged so each partition's data is one contiguous
    # 4KB run (128 big DMA descriptors instead of 1024 x 512B)
    xqT = nc.dram_tensor("xqT", [D, T], FP16, kind="ExternalInput").ap()
    xkT = nc.dram_tensor("xkT", [D, T], FP16, kind="ExternalInput").ap()
    xvT = nc.dram_tensor("xvT", [D, T], FP16, kind="ExternalInput").ap()
    wqT = nc.dram_tensor("wqT", [P, NKC, HC], FP16, kind="ExternalInput").ap()
    wkT = nc.dram_tensor("wkT", [P, 2, NKC, P], FP16, kind="ExternalInput").ap()
    wvT = nc.dram_tensor("wvT", [P, NKC, HC], FP16, kind="ExternalInput").ap()
    woT = nc.dram_tensor("woT", [P, HC // P, D], FP16, kind="ExternalInput").ap()
    bqv = nc.dram_tensor("bqv", [P, HC // P], F32, kind="ExternalInput").ap()
    bkv = nc.dram_tensor("bkv", [P, HC // P], F32, kind="ExternalInput").ap()
    out = nc.dram_tensor("out", [T, D], FP16, kind="ExternalOutput").ap()

    with tile.TileContext(nc) as tc:
        kernel_body(tc, xqT, xkT, xvT, wqT, wkT, wvT, woT, bqv, bkv, out)

    nc.compile()
    return nc


def kernel_body(tc, xqT, xkT, xvT, wqT, wkT, wvT, woT, bqv, bkv, out):
    nc = tc.nc
    NKC = D // P   # 8 contraction chunks for projections
    NXP = NKC // 2  # 4 kc-pair x tiles per (tensor, tb)

    with (
        tc.tile_pool(name="consts", bufs=1) as consts,
        tc.tile_pool(name="xs", bufs=14) as xs,
        tc.tile_pool(name="persist", bufs=1) as persist,
        tc.tile_pool(name="attn", bufs=6) as attn_pool,
        tc.tile_pool(name="small", bufs=4) as small,
        tc.tile_pool(name="outs", bufs=4) as outs,
        tc.tile_pool(name="ps_mm", bufs=2, space="PSUM") as ps_mm,
        tc.tile_pool(name="ps_sc", bufs=2, space="PSUM") as ps_sc,
        tc.tile_pool(name="ps_pv", bufs=2, space="PSUM") as ps_pv,
    ):
        # ------------- x tiles (DVE DMA queue — parallel to SP) -------------
        _xid = [0]

        def x_load(xT, tb, tag):
            """Load one token block's 8 kc chunks as 4 kc-pair tiles."""
            _xid[0] += 1
            ts = []
            for j in range(NXP):
                t = xs.tile([P, 2, QB], FP16, tag="x", name=f"{tag}_{_xid[0]}_{j}")
                nc.gpsimd.dma_start(
                    t,
                    xT.rearrange("(kc p) t -> p kc t", p=P)[
                        :, 2 * j : 2 * j + 2, tb * QB : (tb + 1) * QB
                    ],
                )
                ts.append(t)
            return ts

        def x_ap(xts, kc):
            return xts[kc // 2][:, kc % 2, :]

        # ------------- constants (SP DMA queue) -------------
        # wk arrives m-chunk-major and lands as two half DMAs so the very
        # first projection matmul only waits for the m0 half.
        wk_s = consts.tile([P, 2, NKC, P], FP16)
        nc.sync.dma_start(wk_s[:, 0], wkT[:, 0])
        nc.sync.dma_start(wk_s[:, 1], wkT[:, 1])
        bk_s = consts.tile([P, HC // P], F32)
        nc.sync.dma_start(bk_s, bkv)

        # ------------- persistent activations -------------
        QT = [persist.tile([P, T], FP16, name=f"QT{m}") for m in range(2)]
        KT = [persist.tile([P, T], FP16, name=f"KT{m}") for m in range(2)]
        V = persist.tile([P, NKB, HL * VW], FP16, name="V")
        AC = [persist.tile([P, T], FP16, name=f"AC{c}") for c in range(2)]

        # whole-tile memset: evacs overwrite the value cols, ones cols stay 1.0
        nc.vector.memset(V, 1.0)

        # ------------- projection emitters -------------
        def wk_ap(kc, m):
            return wk_s[:, m, kc, :]

        def wq_ap(kc, m):
            return wq_s[:, kc, m * P : (m + 1) * P]

        def proj_qk(xts, w_ap, b_s, dst, tag, tb, m):
            ps = ps_mm.tile([P, QB], F32, tag="mm", name=f"p_{tag}{m}{tb}")
            for kc in range(NKC):
                nc.tensor.matmul(
                    ps,
                    lhsT=w_ap(kc, m),
                    rhs=x_ap(xts, kc),
                    start=(kc == 0),
                    stop=(kc == NKC - 1),
                )
            nc.vector.tensor_scalar_add(
                dst[m][:, tb * QB : (tb + 1) * QB], ps, b_s[:, m : m + 1]
            )

        def proj_v_block(xts, tb, i):
            t128 = tb * (QB // P) + i
            ps = ps_mm.tile([P, HC], F32, tag="mm", name=f"p_v{t128}")
            for kc in range(NKC):
                nc.tensor.matmul(
                    ps,
                    lhsT=x_ap(xts, kc)[:, i * P : (i + 1) * P],
                    rhs=wv_s[:, kc, :],
                    start=(kc == 0),
                    stop=(kc == NKC - 1),
                )
            nc.vector.tensor_copy(
                V[:, t128].rearrange("p (h c) -> p h c", c=VW)[:, :, DK:],
                ps.rearrange("p (h c) -> p h c", c=DK),
            )

        def load_unit(xT, tb, tag, st):
            return lambda: st.__setitem__("x", x_load(xT, tb, tag))

        def qk_job_units(w_ap, b_s, dst, tag, tb, m, st):
            """8 matmul units + evac for one (m, tb) projection job."""
            units = []
            for kc in range(NKC):

                def mk_mm(m=m, kc=kc, st=st, tb=tb):
                    if kc == 0:
                        st["ps", m] = ps_mm.tile(
                            [P, QB], F32, tag="mm", name=f"pz_{tag}{m}{tb}"
                        )
                    nc.tensor.matmul(
                        st["ps", m],
                        lhsT=w_ap(kc, m),
                        rhs=x_ap(st["x"], kc),
                        start=(kc == 0),
                        stop=(kc == NKC - 1),
                    )

                units.append(mk_mm)

            def mk_evac(m=m, st=st, tb=tb):
                nc.vector.tensor_scalar_add(
                    dst[m][:, tb * QB : (tb + 1) * QB],
                    st["ps", m],
                    b_s[:, m : m + 1],
                )

            units.append(mk_evac)
            return units

        def v_block_units(tb, st):
            return [
                (lambda i=i, st=st, tb=tb: proj_v_block(st["x"], tb, i))
                for i in range(QB // P)
            ]

        # ------------- stage A: K/V/Q for token blocks 0-1 -------------
        # Covers attention kb 0..7 and q blocks 0..1; tb2-3 + the output
        # projections drain through the attention kb loop.
        xk01 = {tb: x_load(xkT, tb, "xk") for tb in (0, 1)}
        wv_s = consts.tile([P, NKC, HC], FP16)
        nc.sync.dma_start(wv_s, wvT)
        wq_s = consts.tile([P, NKC, HC], FP16)
        nc.sync.dma_start(wq_s, wqT)
        bq_s = consts.tile([P, HC // P], F32)
        nc.sync.dma_start(bq_s, bqv)
        for m in (0, 1):
            for tb in (0, 1):
                proj_qk(xk01[tb], wk_ap, bk_s, KT, "xk", tb, m)
        xv01 = {tb: x_load(xvT, tb, "xv") for tb in (0, 1)}
        for tb in (0, 1):
            for i in range(QB // P):
                proj_v_block(xv01[tb], tb, i)
        xq01 = {tb: x_load(xqT, tb, "xq") for tb in (0, 1)}
        wo_s = consts.tile([P, HC // P, D], FP16)
        nc.sync.dma_start(wo_s, woT)
        for m in (0, 1):
            for tb in (0, 1):
                proj_qk(xq01[tb], wq_ap, bq_s, QT, "xq", tb, m)

        # remaining projections, ordered by when attention needs them:
        # tb2 by kb8, tb3 by kb12 (m0/hp0), m1 chunks by kb16+ (hp1),
        # Q tb2/tb3 by qb2/qb3.
        # x loads lead their consumers by a few units so the Pool-queue DMA
        # latency (~2-3us for a 4-tile burst) is hidden.
        stK2, stV2, stK3, stV3, stQ2, stQ3 = ({} for _ in range(6))
        zip_units = (
            [load_unit(xkT, 2, "xk", stK2), load_unit(xvT, 2, "xv", stV2)]
            + qk_job_units(wk_ap, bk_s, KT, "xk", 2, 0, stK2)
            + v_block_units(2, stV2)
            + [load_unit(xkT, 3, "xk", stK3)]
            + qk_job_units(wk_ap, bk_s, KT, "xk", 2, 1, stK2)
            + [load_unit(xvT, 3, "xv", stV3)]
            + qk_job_units(wk_ap, bk_s, KT, "xk", 3, 0, stK3)
            + v_block_units(3, stV3)
            + [load_unit(xqT, 2, "xq", stQ2)]
            + qk_job_units(wk_ap, bk_s, KT, "xk", 3, 1, stK3)
            + qk_job_units(wq_ap, bq_s, QT, "xq", 2, 0, stQ2)
            + qk_job_units(wq_ap, bq_s, QT, "xq", 2, 1, stQ2)
            + [load_unit(xqT, 3, "xq", stQ3)]
            + qk_job_units(wq_ap, bq_s, QT, "xq", 3, 0, stQ3)
            + qk_job_units(wq_ap, bq_s, QT, "xq", 3, 1, stQ3)
        )
        zq = list(zip_units)[::-1]  # pop from end

        def drain(n):
            for _ in range(n):
                if zq:
                    zq.pop()()

        def oproj_units(qb):
            """Output projection for one q block as drainable units."""
            units = []
            for i in range(QB // P):
                t128 = qb * (QB // P) + i
                st = {}

                def mk_mm(t128=t128, st=st, n=0):
                    ps = ps_mm.tile([P, 512], F32, tag="mm", name=f"ps_o{t128}{n}")
                    for c in range(2):
                        nc.tensor.matmul(
                            ps,
                            lhsT=AC[c][:, t128 * P : (t128 + 1) * P],
                            rhs=wo_s[:, c, n * 512 : (n + 1) * 512],
                            start=(c == 0),
                            stop=(c == 1),
                        )
                    st[n] = ps

                def mk_out(t128=t128, st=st, n=0):
                    ob = outs.tile([P, 512], FP16, tag="ob", name=f"ob_{t128}_{n}")
                    nc.vector.tensor_copy(ob, st[n])
                    nc.sync.dma_start(
                        out[t128 * P : (t128 + 1) * P, n * 512 : (n + 1) * 512], ob
                    )

                for n in range(2):
                    units.append(lambda f=mk_mm, n=n: f(n=n))
                    units.append(lambda f=mk_out, n=n: f(n=n))
            return units

        # ---------------- attention ----------------
        # Head pairs (2*hp, 2*hp+1) run their score matmuls concurrently on
        # disjoint PE row groups (K=64 each, base partitions 0 / 64).
        #
        # One flat software pipeline across all (qb, hp) iterations with a
        # 2-kb lag: PV for block kb issues after the score pair for kb+2, so
        # when the PE reaches a pv pair its exp has long completed — the PE
        # stream never stalls on a just-finished exp (ScalarE stays the
        # steady-state limiter), including across hp/qb boundaries. The
        # normalize for an hp and the O-projection hand-off ride the flush
        # of that hp's last k block, two slots into the next hp.
        def emit_pv(e):
            qb, hp, kb, at, pv0, pv1 = e
            m, h0, h1 = hp, 2 * hp, 2 * hp + 1
            nc.tensor.matmul(
                pv0,
                lhsT=V[:, kb, VW * h0 : VW * (h0 + 1)],
                rhs=at[:, :QB],
                start=(kb == 0),
                stop=(kb == NKB - 1),
            )
            nc.tensor.matmul(
                pv1,
                lhsT=V[:, kb, VW * h1 : VW * (h1 + 1)],
                rhs=at[:, QB:],
                start=(kb == 0),
                stop=(kb == NKB - 1),
            )
            if kb == NKB - 1:
                for h, pv in ((h0, pv0), (h1, pv1)):
                    off = 64 * (h % 2)
                    # rows 0-63 of pv: softmax denominator replicated across
                    # 64 partitions (ones cols of V); rows 64-127: the head
                    # values. One DVE fast-reciprocal + one multiply from
                    # PSUM. (The custom-DVE reciprocal ignores input
                    # partition offsets — its input must sit at partition 0.)
                    rcp_bc = small.tile(
                        [DK, QB], F32, tag="rcp", name=f"rcp_{qb}_{h}"
                    )
                    nc.vector.reciprocal_approx_fast(rcp_bc, pv[:DK, :])
                    nc.vector.tensor_mul(
                        AC[m][off : off + DK, qb * QB : (qb + 1) * QB],
                        pv[DK:P, :],
                        rcp_bc,
                    )
                if hp == 1:
                    # this qb's AC is complete: queue its output projection
                    zq.extend(oproj_units(qb)[::-1])

        pend = []
        for qb in range(NQB):
            for hp in range(2):
                m = hp  # heads (2*hp, 2*hp+1) live in QT/KT chunk m
                pv0 = ps_pv.tile([P, QB], F32, tag="pv", name=f"pv_{qb}_{hp}0")
                pv1 = ps_pv.tile([P, QB], F32, tag="pv", name=f"pv_{qb}_{hp}1")
                for kb in range(NKB):
                    sc = ps_sc.tile(
                        [P, 2 * QB], F32, tag="sc", name=f"sc_{qb}_{hp}_{kb}"
                    )
                    nc.tensor.matmul(
                        sc[:, :QB],
                        lhsT=KT[m][0:DK, kb * P : (kb + 1) * P],
                        rhs=QT[m][0:DK, qb * QB : (qb + 1) * QB],
                        start=True,
                        stop=True,
                    )
                    nc.tensor.matmul(
                        sc[:, QB:],
                        lhsT=KT[m][DK:P, kb * P : (kb + 1) * P],
                        rhs=QT[m][DK:P, qb * QB : (qb + 1) * QB],
                        start=True,
                        stop=True,
                    )
                    at = attn_pool.tile(
                        [P, 2 * QB], FP16, tag="at", name=f"at_{qb}_{hp}_{kb}"
                    )
                    nc.scalar.activation(at, sc, AF.Exp, scale=0.125)
                    pend.append((qb, hp, kb, at, pv0, pv1))
                    if len(pend) > 2:
                        emit_pv(pend.pop(0))
                    drain(5 if qb == 0 else 1)
        for e in pend:
            emit_pv(e)

        drain(len(zip_units) + 64)


_module_cache = None


def get_module():
    global _module_cache
    if _module_cache is None:
        _module_cache = build_module()
    return _module_cache


def shard_inputs(query, key, value, Wq, bq, Wk, bk, Wv, bv, Wo, bo):
    """Build the 8 per-core input maps (host-side layout transforms only)."""
    f = np.float32
    h = np.float16
    xT = {}
    for b in range(B):
        xT["q", b] = np.ascontiguousarray(np.asarray(query, f)[:, b, :].T.astype(h))
        xT["k", b] = np.ascontiguousarray(np.asarray(key, f)[:, b, :].T.astype(h))
        xT["v", b] = np.ascontiguousarray(np.asarray(value, f)[:, b, :].T.astype(h))
    Wq, Wk, Wv, Wo = (np.asarray(w, f) for w in (Wq, Wk, Wv, Wo))
    bq, bk = np.asarray(bq, f), np.asarray(bk, f)

    def w_arr(WT):
        # [D, HC] -> [P, NKC, HC]: partition-contiguous for big DMA descriptors
        kc = WT.shape[0] // P
        return np.ascontiguousarray(WT.reshape(kc, P, -1).transpose(1, 0, 2).astype(h))

    def b_arr(bv_):
        return np.ascontiguousarray(bv_.reshape(-1, P).T)

    in_maps = []
    for c in range(NCORES):
        b, hg = c // (NCORES // B), c % (NCORES // B)
        cols = slice(HC * hg, HC * (hg + 1))
        in_maps.append(
            {
                "xqT": xT["q", b],
                "xkT": xT["k", b],
                "xvT": xT["v", b],
                "wqT": w_arr(Wq[cols, :].T),
                "wkT": np.ascontiguousarray(
                    Wk[cols, :].T.reshape(8, P, 2, P).transpose(1, 2, 0, 3).astype(h)
                ),
                "wvT": w_arr(Wv[cols, :].T),
                "woT": w_arr(Wo[:, cols].T),
                "bqv": b_arr(bq[cols]),
                "bkv": b_arr(bk[cols]),
            }
        )
    return in_maps


def kernel(query, key, value, Wq, bq, Wk, bk, Wv, bv, Wo, bo, trace=False):
    nc = get_module()
    in_maps = shard_inputs(query, key, value, Wq, bq, Wk, bk, Wv, bv, Wo, bo)
    res = bass_utils.run_bass_kernel_spmd(
        nc, in_maps, core_ids=list(range(NCORES)), trace=trace
    )
    f = np.float32
    bias_term = np.asarray(bv, f) @ np.asarray(Wo, f).T + np.asarray(bo, f)
    output = np.empty((S, B, D), f)
    for b in range(B):
        acc = res.results[4 * b]["out"].astype(f)
        for c in range(4 * b + 1, 4 * b + 4):
            acc = acc + res.results[c]["out"].astype(f)
        output[:, b, :] = acc + bias_term
    if trace:
        kernel.last_results = res
    return output


# revision 36
# speedup vs baseline: 1.0269x; 1.0064x over previous
"""MultiHeadAttention TRN2 Bass kernel (v4, fp16 datapath).

Problem: S=2048, B=2, H=16, d_k=64, D=1024, fp32 interface.
  q = query @ Wq.T + bq ; k = key @ Wk.T + bk ; v = value @ Wv.T + bv
  score = einsum('qbhd,kbhd->qkbh', q, k) / 8 ; attn = softmax(score, axis=k)
  out = einsum('qkbh,kbhd->qbhd', attn, v) -> reshape -> @ Wo.T + bo

Sharding (8 cores): core c handles batch b = c//4 and heads [4*(c%4), 4*(c%4)+4).
Each core computes its partial output projection (tensor-parallel along the
head dim); the host sums the 4 partials per batch and adds the bias terms
(bv @ Wo.T + bo, the linear-foldable bias contributions).

Key techniques (see git-style history in the module docstrings of prior
versions):
  * fp16 device datapath (host casts inputs/weights; DMA bytes halve; PE
    speed identical to f32r at 1 cycle/row; PSUM accumulates fp32).
  * V head slots are 128 cols: [64 replicated ones | 64 value dims]. The PV
    matmul cost is free-dim bound, so the otherwise idle M columns produce
    the softmax denominator pre-broadcast across 64 partitions for free.
    Ones first: custom-DVE ops ignore input partition offsets, so the
    denominator must sit at partition 0 for the fast reciprocal.
  * Softmax normalize fully on DVE straight from PSUM: fast-reciprocal of
    pv[0:64] then one multiply into fp16 AC.
  * Score matmuls for a head pair use disjoint PE partition halves (K=64 at
    base 0 / 64) and co-run on the PE array.
  * DMA: weights ride the SP queue as whole-tensor transfers; x tiles ride
    the DVE queue (a parallel HWDGE issue stream — each dma_start costs
    ~0.65us of queue time, so a single queue serializes startup).
  * The kb loop emits PV(kb-1) before scores(kb) so the PE stream alternates
    deterministically; exp(kb) on ScalarE is the steady-state limiter.
  * Each qb's output projection drains through the next qb's kb loop (the
    drain queue) instead of bursting at the qb boundary.

Per-core device layout (host pre-transposes + casts, pure data layout):
  xqT/xkT/xvT : [D=1024, T=2048] fp16   input slices, feature-major
  wqT/wkT/wvT : [1024, 256] fp16        Wq[rows,:].T etc (lhsT tiles directly)
  woT         : [256, 1024] fp16        Wo[:, cols].T (rhs tiles directly)
  bqv/bkv     : [256] fp32              projection biases for q/k
  out         : [2048, 1024] fp16       partial output (token-major)
"""

import os

os.environ.setdefault("MYCRO_LOCAL_CACHE", "1")

import numpy as np

import concourse.bass as bass
import concourse.tile as tile
from concourse import bacc, bass_utils, mybir


def _install_ntff_hook():
    """Provide antenv.axon_hooks when the image lacks it, so trace=True can
    capture NTFF profiles through the axon tunnel. Degrades silently."""
    import contextlib
    import ctypes
    import sys

    if "antenv.axon_hooks" in sys.modules:
        return
    so_path = "/opt/axon/libaxon_pjrt.so"
    if not os.path.exists(so_path):
        return
    try:
        lib = ctypes.CDLL(so_path)
        if not hasattr(lib, "axon_start_nrt_profile"):
            return
        lib.axon_start_nrt_profile.argtypes = [
            ctypes.POINTER(ctypes.c_int64),
            ctypes.c_size_t,
        ]
        lib.axon_start_nrt_profile.restype = ctypes.c_int64
        lib.axon_stop_nrt_profile.argtypes = [ctypes.c_char_p]
        lib.axon_stop_nrt_profile.restype = ctypes.c_int64

        @contextlib.contextmanager
        def _hook(output_dir, device_ids):
            import jax

            jax.devices()
            if device_ids:
                ids = (ctypes.c_int64 * len(device_ids))(*device_ids)
                rc = lib.axon_start_nrt_profile(ids, len(device_ids))
            else:
                rc = lib.axon_start_nrt_profile(None, 0)
            if rc != 0:
                raise RuntimeError(f"axon_start_nrt_profile rc={rc}")
            try:
                yield
            finally:
                n = lib.axon_stop_nrt_profile(str(output_dir).encode())
                print(f"ntff profile: {n} file(s) -> {output_dir}")

        import types

        mod = types.ModuleType("antenv.axon_hooks")
        mod.get_axon_ntff_profile_hook = lambda: _hook
        mod.set_axon_ntff_profile_hook = lambda h: None
        sys.modules["antenv.axon_hooks"] = mod
    except Exception:
        pass


_install_ntff_hook()

F32 = mybir.dt.float32
FP16 = mybir.dt.float16
AF = mybir.ActivationFunctionType

S = 2048          # sequence length
B = 2             # batch
H = 16            # total heads
DK = 64           # head dim
D = 1024          # model dim
NCORES = 8
HL = H // (NCORES // B)   # heads per core = 4
HC = HL * DK              # head cols per core = 256
T = S                     # tokens per core (one batch element)
P = 128
QB = 512                  # q block (matmul free dim)
NKB = T // P              # 16 k blocks
NQB = T // QB             # 4 q blocks
VW = 2 * DK               # 128: [64 ones | 64 value dims] per head slot


def build_module():
    nc = bacc.Bacc("TRN2", target_bir_lowering=False, debug=False)

    NKC = D // P
    # weights arrive pre-arranged so each partition's data is one contiguous
    # 4KB run (128 big DMA descriptors instead of 1024 x 512B)
    xqT = nc.dram_tensor("xqT", [D, T], FP16, kind="ExternalInput").ap()
    xkT = nc.dram_tensor("xkT", [D, T], FP16, kind="ExternalInput").ap()
    xvT = nc.dram_tensor("xvT", [D, T], FP16, kind="ExternalInput").ap()
    wqT = nc.dram_tensor("wqT", [P, NKC, HC], FP16, kind="ExternalInput").ap()
    wkT = nc.dram_tensor("wkT", [P, 2, NKC, P], FP16, kind="ExternalInput").ap()
    wvT = nc.dram_tensor("wvT", [P, NKC, HC], FP16, kind="ExternalInput").ap()
    woT = nc.dram_tensor("woT", [P, HC // P, D], FP16, kind="ExternalInput").ap()
    bqv = nc.dram_tensor("bqv", [P, HC // P], F32, kind="ExternalInput").ap()
    bkv = nc.dram_tensor("bkv", [P, HC // P], F32, kind="ExternalInput").ap()
    out = nc.dram_tensor("out", [T, D], FP16, kind="ExternalOutput").ap()

    with tile.TileContext(nc) as tc:
        kernel_body(tc, xqT, xkT, xvT, wqT, wkT, wvT, woT, bqv, bkv, out)

    nc.compile()
    return nc


def kernel_body(tc, xqT, xkT, xvT, wqT, wkT, wvT, woT, bqv, bkv, out):
    nc = tc.nc
    NKC = D // P   # 8 contraction chunks for projections
    NXP = NKC // 2  # 4 kc-pair x tiles per (tensor, tb)

    with (
        tc.tile_pool(name="consts", bufs=1) as consts,
        tc.tile_pool(name="xs", bufs=14) as xs,
        tc.tile_pool(name="persist", bufs=1) as persist,
        tc.tile_pool(name="attn", bufs=6) as attn_pool,
        tc.tile_pool(name="small", bufs=4) as small,
        tc.tile_pool(name="outs", bufs=4) as outs,
        tc.tile_pool(name="ps_mm", bufs=2, space="PSUM") as ps_mm,
        tc.tile_pool(name="ps_sc", bufs=2, space="PSUM") as ps_sc,
        tc.tile_pool(name="ps_pv", bufs=2, space="PSUM") as ps_pv,
    ):
        # ------------- x tiles (DVE DMA queue — parallel to SP) -------------
        _xid = [0]

        def x_load(xT, tb, tag):
            """Load one token block's 8 kc chunks as 4 kc-pair tiles."""
            _xid[0] += 1
            ts = []
            for j in range(NXP):
                t = xs.tile([P, 2, QB], FP16, tag="x", name=f"{tag}_{_xid[0]}_{j}")
                nc.gpsimd.dma_start(
                    t,
                    xT.rearrange("(kc p) t -> p kc t", p=P)[
                        :, 2 * j : 2 * j + 2, tb * QB : (tb + 1) * QB
                    ],
                )
                ts.append(t)
            return ts

        def x_ap(xts, kc):
            return xts[kc // 2][:, kc % 2, :]

        # ------------- constants (SP DMA queue) -------------
        # wk arrives m-chunk-major and lands as two half DMAs so the very
        # first projection matmul only waits for the m0 half.
        wk_s = consts.tile([P, 2, NKC, P], FP16)
        nc.sync.dma_start(wk_s[:, 0], wkT[:, 0])
        nc.sync.dma_start(wk_s[:, 1], wkT[:, 1])
        bk_s = consts.tile([P, HC // P], F32)
        nc.sync.dma_start(bk_s, bkv)

        # ------------- persistent activations -------------
        QT = [persist.tile([P, T], FP16, name=f"QT{m}") for m in range(2)]
        KT = [persist.tile([P, T], FP16, name=f"KT{m}") for m in range(2)]
        V = persist.tile([P, NKB, HL * VW], FP16, name="V")
        AC = [persist.tile([P, T], FP16, name=f"AC{c}") for c in range(2)]



        # ------------- projection emitters -------------
        def wk_ap(kc, m):
            return wk_s[:, m, kc, :]

        def wq_ap(kc, m):
            return wq_s[:, kc, m * P : (m + 1) * P]

        def proj_qk(xts, w_ap, b_s, dst, tag, tb, m):
            ps = ps_mm.tile([P, QB], F32, tag="mm", name=f"p_{tag}{m}{tb}")
            for kc in range(NKC):
                nc.tensor.matmul(
                    ps,
                    lhsT=w_ap(kc, m),
                    rhs=x_ap(xts, kc),
                    start=(kc == 0),
                    stop=(kc == NKC - 1),
                )
            nc.vector.tensor_scalar_add(
                dst[m][:, tb * QB : (tb + 1) * QB], ps, b_s[:, m : m + 1]
            )

        def proj_v_block(xts, tb, i):
            t128 = tb * (QB // P) + i
            ps = ps_mm.tile([P, HC], F32, tag="mm", name=f"p_v{t128}")
            for kc in range(NKC):
                nc.tensor.matmul(
                    ps,
                    lhsT=x_ap(xts, kc)[:, i * P : (i + 1) * P],
                    rhs=wv_s[:, kc, :],
                    start=(kc == 0),
                    stop=(kc == NKC - 1),
                )
            nc.vector.tensor_copy(
                V[:, t128].rearrange("p (h c) -> p h c", c=VW)[:, :, DK:],
                ps.rearrange("p (h c) -> p h c", c=DK),
            )

        def load_unit(xT, tb, tag, st):
            return lambda: st.__setitem__("x", x_load(xT, tb, tag))

        def qk_job_units(w_ap, b_s, dst, tag, tb, m, st):
            """8 matmul units + evac for one (m, tb) projection job."""
            units = []
            for kc in range(NKC):

                def mk_mm(m=m, kc=kc, st=st, tb=tb):
                    if kc == 0:
                        st["ps", m] = ps_mm.tile(
                            [P, QB], F32, tag="mm", name=f"pz_{tag}{m}{tb}"
                        )
                    nc.tensor.matmul(
                        st["ps", m],
                        lhsT=w_ap(kc, m),
                        rhs=x_ap(st["x"], kc),
                        start=(kc == 0),
                        stop=(kc == NKC - 1),
                    )

                units.append(mk_mm)

            def mk_evac(m=m, st=st, tb=tb):
                nc.vector.tensor_scalar_add(
                    dst[m][:, tb * QB : (tb + 1) * QB],
                    st["ps", m],
                    b_s[:, m : m + 1],
                )

            units.append(mk_evac)
            return units

        def v_block_units(tb, st):
            return [
                (lambda i=i, st=st, tb=tb: proj_v_block(st["x"], tb, i))
                for i in range(QB // P)
            ]

        # ------------- stage A: K/V/Q for token blocks 0-1 -------------
        # Covers attention kb 0..7 and q blocks 0..1; tb2-3 + the output
        # projections drain through the attention kb loop.
        xk01 = {tb: x_load(xkT, tb, "xk") for tb in (0, 1)}
        wv_s = consts.tile([P, NKC, HC], FP16)
        nc.sync.dma_start(wv_s, wvT)
        wq_s = consts.tile([P, NKC, HC], FP16)
        nc.sync.dma_start(wq_s, wqT)
        bq_s = consts.tile([P, HC // P], F32)
        nc.sync.dma_start(bq_s, bqv)
        for m in (0, 1):
            for tb in (0, 1):
                proj_qk(xk01[tb], wk_ap, bk_s, KT, "xk", tb, m)
        # ones columns of V (denominator trick); emitted after the K projs so
        # their PSUM evacs aren't queued behind it on DVE, and strided so only
        # the ones cols are touched (the value cols get written by the evacs)
        nc.vector.memset(
            V.rearrange("p t (h c) -> p t h c", c=VW)[:, :, :, :DK], 1.0
        )
        xv01 = {tb: x_load(xvT, tb, "xv") for tb in (0, 1)}
        for tb in (0, 1):
            for i in range(QB // P):
                proj_v_block(xv01[tb], tb, i)
        xq01 = {tb: x_load(xqT, tb, "xq") for tb in (0, 1)}
        wo_s = consts.tile([P, HC // P, D], FP16)
        nc.sync.dma_start(wo_s, woT)
        for m in (0, 1):
            for tb in (0, 1):
                proj_qk(xq01[tb], wq_ap, bq_s, QT, "xq", tb, m)

        # remaining projections, ordered by when attention needs them:
        # tb2 by kb8, tb3 by kb12 (m0/hp0), m1 chunks by kb16+ (hp1),
        # Q tb2/tb3 by qb2/qb3.
        # x loads lead their consumers by a few units so the Pool-queue DMA
        # latency (~2-3us for a 4-tile burst) is hidden.
        stK2, stV2, stK3, stV3, stQ2, stQ3 = ({} for _ in range(6))
        zip_units = (
            [load_unit(xkT, 2, "xk", stK2), load_unit(xvT, 2, "xv", stV2)]
            + qk_job_units(wk_ap, bk_s, KT, "xk", 2, 0, stK2)
            + v_block_units(2, stV2)
            + [load_unit(xkT, 3, "xk", stK3)]
            + qk_job_units(wk_ap, bk_s, KT, "xk", 2, 1, stK2)
            + [load_unit(xvT, 3, "xv", stV3)]
            + qk_job_units(wk_ap, bk_s, KT, "xk", 3, 0, stK3)
            + v_block_units(3, stV3)
            + [load_unit(xqT, 2, "xq", stQ2)]
            + qk_job_units(wk_ap, bk_s, KT, "xk", 3, 1, stK3)
            + qk_job_units(wq_ap, bq_s, QT, "xq", 2, 0, stQ2)
            + qk_job_units(wq_ap, bq_s, QT, "xq", 2, 1, stQ2)
            + [load_unit(xqT, 3, "xq", stQ3)]
            + qk_job_units(wq_ap, bq_s, QT, "xq", 3, 0, stQ3)
            + qk_job_units(wq_ap, bq_s, QT, "xq", 3, 1, stQ3)
        )
        zq = list(zip_units)[::-1]  # pop from end

        def drain(n):
            for _ in range(n):
                if zq:
                    zq.pop()()

        def oproj_units(qb):
            """Output projection for one q block as drainable units."""
            units = []
            for i in range(QB // P):
                t128 = qb * (QB // P) + i
                st = {}

                def mk_mm(t128=t128, st=st, n=0):
                    ps = ps_mm.tile([P, 512], F32, tag="mm", name=f"ps_o{t128}{n}")
                    for c in range(2):
                        nc.tensor.matmul(
                            ps,
                            lhsT=AC[c][:, t128 * P : (t128 + 1) * P],
                            rhs=wo_s[:, c, n * 512 : (n + 1) * 512],
                            start=(c == 0),
                            stop=(c == 1),
                        )
                    st[n] = ps

                def mk_out(t128=t128, st=st, n=0):
                    ob = outs.tile([P, 512], FP16, tag="ob", name=f"ob_{t128}_{n}")
                    nc.vector.tensor_copy(ob, st[n])
                    nc.sync.dma_start(
                        out[t128 * P : (t128 + 1) * P, n * 512 : (n + 1) * 512], ob
                    )

                for n in range(2):
                    units.append(lambda f=mk_mm, n=n: f(n=n))
                    units.append(lambda f=mk_out, n=n: f(n=n))
            return units

        # ---------------- attention ----------------
        # Head pairs (2*hp, 2*hp+1) run their score matmuls concurrently on
        # disjoint PE row groups (K=64 each, base partitions 0 / 64).
        #
        # One flat software pipeline across all (qb, hp) iterations with a
        # 2-kb lag: PV for block kb issues after the score pair for kb+2, so
        # when the PE reaches a pv pair its exp has long completed — the PE
        # stream never stalls on a just-finished exp (ScalarE stays the
        # steady-state limiter), including across hp/qb boundaries. The
        # normalize for an hp and the O-projection hand-off ride the flush
        # of that hp's last k block, two slots into the next hp.
        def emit_pv(e):
            qb, hp, kb, at, pv0, pv1 = e
            m, h0, h1 = hp, 2 * hp, 2 * hp + 1
            nc.tensor.matmul(
                pv0,
                lhsT=V[:, kb, VW * h0 : VW * (h0 + 1)],
                rhs=at[:, :QB],
                start=(kb == 0),
                stop=(kb == NKB - 1),
            )
            nc.tensor.matmul(
                pv1,
                lhsT=V[:, kb, VW * h1 : VW * (h1 + 1)],
                rhs=at[:, QB:],
                start=(kb == 0),
                stop=(kb == NKB - 1),
            )
            if kb == NKB - 1:
                for h, pv in ((h0, pv0), (h1, pv1)):
                    off = 64 * (h % 2)
                    # rows 0-63 of pv: softmax denominator replicated across
                    # 64 partitions (ones cols of V); rows 64-127: the head
                    # values. One DVE fast-reciprocal + one multiply from
                    # PSUM. (The custom-DVE reciprocal ignores input
                    # partition offsets — its input must sit at partition 0.)
                    rcp_bc = small.tile(
                        [DK, QB], F32, tag="rcp", name=f"rcp_{qb}_{h}"
                    )
                    nc.vector.reciprocal_approx_fast(rcp_bc, pv[:DK, :])
                    nc.vector.tensor_mul(
                        AC[m][off : off + DK, qb * QB : (qb + 1) * QB],
                        pv[DK:P, :],
                        rcp_bc,
                    )
                if hp == 1:
                    # this qb's AC is complete: queue its output projection
                    zq.extend(oproj_units(qb)[::-1])

        pend = []
        for qb in range(NQB):
            for hp in range(2):
                m = hp  # heads (2*hp, 2*hp+1) live in QT/KT chunk m
                pv0 = ps_pv.tile([P, QB], F32, tag="pv", name=f"pv_{qb}_{hp}0")
                pv1 = ps_pv.tile([P, QB], F32, tag="pv", name=f"pv_{qb}_{hp}1")
                for kb in range(NKB):
                    sc = ps_sc.tile(
                        [P, 2 * QB], F32, tag="sc", name=f"sc_{qb}_{hp}_{kb}"
                    )
                    nc.tensor.matmul(
                        sc[:, :QB],
                        lhsT=KT[m][0:DK, kb * P : (kb + 1) * P],
                        rhs=QT[m][0:DK, qb * QB : (qb + 1) * QB],
                        start=True,
                        stop=True,
                    )
                    nc.tensor.matmul(
                        sc[:, QB:],
                        lhsT=KT[m][DK:P, kb * P : (kb + 1) * P],
                        rhs=QT[m][DK:P, qb * QB : (qb + 1) * QB],
                        start=True,
                        stop=True,
                    )
                    at = attn_pool.tile(
                        [P, 2 * QB], FP16, tag="at", name=f"at_{qb}_{hp}_{kb}"
                    )
                    nc.scalar.activation(at, sc, AF.Exp, scale=0.125)
                    pend.append((qb, hp, kb, at, pv0, pv1))
                    if len(pend) > 2:
                        emit_pv(pend.pop(0))
                    drain(5 if qb == 0 else 1)
        for e in pend:
            emit_pv(e)

        drain(len(zip_units) + 64)


_module_cache = None


def get_module():
    global _module_cache
    if _module_cache is None:
        _module_cache = build_module()
    return _module_cache


def shard_inputs(query, key, value, Wq, bq, Wk, bk, Wv, bv, Wo, bo):
    """Build the 8 per-core input maps (host-side layout transforms only)."""
    f = np.float32
    h = np.float16
    xT = {}
    for b in range(B):
        xT["q", b] = np.ascontiguousarray(np.asarray(query, f)[:, b, :].T.astype(h))
        xT["k", b] = np.ascontiguousarray(np.asarray(key, f)[:, b, :].T.astype(h))
        xT["v", b] = np.ascontiguousarray(np.asarray(value, f)[:, b, :].T.astype(h))
    Wq, Wk, Wv, Wo = (np.asarray(w, f) for w in (Wq, Wk, Wv, Wo))
    bq, bk = np.asarray(bq, f), np.asarray(bk, f)

    def w_arr(WT):
        # [D, HC] -> [P, NKC, HC]: partition-contiguous for big DMA descriptors
        kc = WT.shape[0] // P
        return np.ascontiguousarray(WT.reshape(kc, P, -1).transpose(1, 0, 2).astype(h))

    def b_arr(bv_):
        return np.ascontiguousarray(bv_.reshape(-1, P).T)

    in_maps = []
    for c in range(NCORES):
        b, hg = c // (NCORES // B), c % (NCORES // B)
        cols = slice(HC * hg, HC * (hg + 1))
        in_maps.append(
            {
                "xqT": xT["q", b],
                "xkT": xT["k", b],
                "xvT": xT["v", b],
                "wqT": w_arr(Wq[cols, :].T),
                "wkT": np.ascontiguousarray(
                    Wk[cols, :].T.reshape(8, P, 2, P).transpose(1, 2, 0, 3).astype(h)
                ),
                "wvT": w_arr(Wv[cols, :].T),
                "woT": w_arr(Wo[:, cols].T),
                "bqv": b_arr(bq[cols]),
                "bkv": b_arr(bk[cols]),
            }
        )
    return in_maps


def kernel(query, key, value, Wq, bq, Wk, bk, Wv, bv, Wo, bo, trace=False):
    nc = get_module()
    in_maps = shard_inputs(query, key, value, Wq, bq, Wk, bk, Wv, bv, Wo, bo)
    res = bass_utils.run_bass_kernel_spmd(
        nc, in_maps, core_ids=list(range(NCORES)), trace=trace
    )
    f = np.float32
    bias_term = np.asarray(bv, f) @ np.asarray(Wo, f).T + np.asarray(bo, f)
    output = np.empty((S, B, D), f)
    for b in range(B):
        acc = res.results[4 * b]["out"].astype(f)
        for c in range(4 * b + 1, 4 * b + 4):
            acc = acc + res.results[c]["out"].astype(f)
        output[:, b, :] = acc + bias_term
    if trace:
        kernel.last_results = res
    return output
